# revision 1
# baseline (speedup 1.0000x reference)
"""GCN layer on 8 trn2 NeuronCores.

out = segment_sum((h @ W * norm)[src], dst) * norm + bias

Key algebra: (h@W)*norm = (h*norm)@W and segment_sum is linear, so
out = (segment_sum((h*norm)[src], dst) @ W) * norm + bias.
This lets us gather RAW h rows from HBM (no pre-GEMM over all nodes),
apply the edge weight norm[src] inside a one-hot selection matrix, do the
scatter-sum as PSUM-accumulated matmuls, and run the 128x128 weight GEMM
once per output tile after aggregation.

Sharding: nodes padded to 784 tiles of 128; edges partitioned by dst tile.
Tiles are rank-sorted by edge count and dealt across the 8 cores so that
each program slot has near-equal padded edge counts on every core (one
shared SPMD program). h is split into 4 quadrant tables of 25088 rows so
dma_gather's int16 indices can address them.
"""
import numpy as np

import concourse.bass as bass
import concourse.mybir as mybir
import concourse.tile as tile
from concourse import bacc
from concourse.bass_utils import run_bass_kernel_spmd
from concourse.library_config import mlp

P = 128
N = 100000
E = 1600000
NCORES = 8
NT = 784                # node tiles after padding (784*128 = 100352)
NPAD = NT * P
TPC = NT // NCORES      # tiles (slots) per core = 98
Q = 4                   # quadrant tables for int16 gather indices
R = NPAD // Q           # 25088 rows per quadrant

_cache = {}
RUN_KWARGS = {}      # test.py may set {"trace": True} etc.
LAST_RESULTS = None  # BassKernelResults of the last run
EMULATE = False      # numpy emulation of the device program (debug)
LAST_NC = None
LAST_IN_MAPS = None


def _build_program(K_sq, Csq):
    """Build the shared SPMD Bass program.

    K_sq[s][q]: padded (x128) gather count for slot s, quadrant q.
    Csq[s][q]:  K_sq//128 chunks.
    """
    C_s = [sum(Csq[s]) for s in range(TPC)]     # chunks per slot
    Cmax = max(C_s)
    S_s = [sum(K_sq[s]) // 16 for s in range(TPC)]  # idx cols per slot
    idx_cols = sum(S_s)
    chunk_cols = sum(C_s)

    nc = bacc.Bacc(None, target_bir_lowering=False)
    f32 = mybir.dt.float32
    hq_d = [nc.dram_tensor(f"h{q}", [R, P], f32, kind="ExternalInput")
            for q in range(Q)]
    idx_d = nc.dram_tensor("idx16", [P, idx_cols], mybir.dt.int16,
                           kind="ExternalInput")
    dstl_d = nc.dram_tensor("dstl", [P, chunk_cols], f32, kind="ExternalInput")
    ew_d = nc.dram_tensor("ew", [P, chunk_cols], f32, kind="ExternalInput")
    ncol_d = nc.dram_tensor("ncol", [P, TPC], f32, kind="ExternalInput")
    bb_d = nc.dram_tensor("bb", [P, P], f32, kind="ExternalInput")
    w_d = nc.dram_tensor("wt", [P, P], f32, kind="ExternalInput")
    out_d = nc.dram_tensor("out", [TPC * P, P], f32, kind="ExternalOutput")
    out_v = out_d.rearrange("(t p) d -> t p d", p=P)

    with tile.TileContext(nc) as tc:
        with (
            tc.tile_pool(name="const", bufs=1) as cpool,
            tc.tile_pool(name="gather", bufs=2) as gpool,
            tc.tile_pool(name="pt", bufs=4) as ptpool,
            tc.tile_pool(name="ps", bufs=2, space="PSUM") as pspool,
            tc.tile_pool(name="ps2", bufs=2, space="PSUM") as ps2pool,
            tc.tile_pool(name="oo", bufs=3) as opool,
            tc.tile_pool(name="agg", bufs=2) as aggpool,
        ):
            nc.gpsimd.load_library(mlp)
            idx_sb = cpool.tile([P, idx_cols], mybir.dt.int16)
            nc.sync.dma_start(idx_sb[:], idx_d[:])
            dstl_sb = cpool.tile([P, chunk_cols], f32)
            nc.sync.dma_start(dstl_sb[:], dstl_d[:])
            ew_sb = cpool.tile([P, chunk_cols], f32)
            nc.sync.dma_start(ew_sb[:], ew_d[:])
            ncol_sb = cpool.tile([P, TPC], f32)
            nc.sync.dma_start(ncol_sb[:], ncol_d[:])
            bb_sb = cpool.tile([P, P], f32)
            nc.sync.dma_start(bb_sb[:], bb_d[:])
            w_sb = cpool.tile([P, P], f32)
            nc.sync.dma_start(w_sb[:], w_d[:])
            iota_i = cpool.tile([P, P], mybir.dt.int32)
            nc.gpsimd.iota(iota_i[:], pattern=[[1, P]], base=0,
                           channel_multiplier=0)
            iota_f = cpool.tile([P, P], f32)
            nc.vector.tensor_copy(iota_f[:], iota_i[:])

            ioff = 0   # idx col offset
            coff = 0   # chunk col offset
            for s in range(TPC):
                msgs = gpool.tile([P, Cmax, P], f32, tag="msgs")
                local = 0
                io = ioff
                for q in range(Q):
                    K = K_sq[s][q]
                    if K == 0:
                        continue
                    cq = Csq[s][q]
                    nc.gpsimd.dma_gather(
                        msgs[:, local:local + cq, :], hq_d[q][:],
                        idx_sb[:, io:io + K // 16], K, K, P,
                    )
                    local += cq
                    io += K // 16
                aggT_ps = pspool.tile([P, P], f32, tag="agg")
                nchunks = C_s[s]
                for c in range(nchunks):
                    col = coff + c
                    pt = ptpool.tile([P, P], f32, tag="pt")
                    nc.vector.tensor_scalar(
                        pt[:], iota_f[:],
                        dstl_sb[:, col:col + 1],
                        ew_sb[:, col:col + 1],
                        op0=mybir.AluOpType.is_equal,
                        op1=mybir.AluOpType.mult,
                    )
                    nc.tensor.matmul(
                        aggT_ps[:], lhsT=msgs[:, c, :], rhs=pt[:],
                        start=(c == 0), stop=(c == nchunks - 1),
                    )
                aggT_sb = aggpool.tile([P, P], f32, tag="aggT")
                nc.vector.tensor_copy(aggT_sb[:], aggT_ps[:])
                out_ps = ps2pool.tile([P, P], f32, tag="out")
                nc.tensor.matmul(out_ps[:], lhsT=aggT_sb[:], rhs=w_sb[:],
                                 start=True, stop=True)
                o_sb = opool.tile([P, P], f32, tag="o")
                nc.vector.tensor_scalar(
                    o_sb[:], out_ps[:], ncol_sb[:, s:s + 1], None,
                    op0=mybir.AluOpType.mult,
                )
                nc.vector.tensor_tensor(o_sb[:], o_sb[:], bb_sb[:],
                                        op=mybir.AluOpType.add)
                nc.sync.dma_start(out_v[s], o_sb[:])
                ioff += S_s[s]
                coff += nchunks
    nc.compile()
    return nc


def kernel(h, norm, src, dst, weight, bias):
    h = np.ascontiguousarray(h, dtype=np.float32)
    norm = np.ascontiguousarray(norm, dtype=np.float32).reshape(-1)
    src = np.ascontiguousarray(src, dtype=np.int64).reshape(-1)
    dst = np.ascontiguousarray(dst, dtype=np.int64).reshape(-1)
    weight = np.ascontiguousarray(weight, dtype=np.float32)
    bias = np.ascontiguousarray(bias, dtype=np.float32).reshape(-1)
    n, d = h.shape
    e = src.shape[0]
    assert (n, d, e) == (N, P, E), (n, d, e)

    h_pad = np.zeros((NPAD, P), np.float32)
    h_pad[:n] = h
    hq = [np.ascontiguousarray(h_pad[q * R:(q + 1) * R]) for q in range(Q)]
    norm_pad = np.zeros((NPAD,), np.float32)
    norm_pad[:n] = norm

    tile_id = dst // P
    dstl_all = (dst % P).astype(np.float32)
    quad = src // R
    srcl_all = (src % R).astype(np.int16)
    ew_all = norm[src].astype(np.float32)

    key = tile_id * Q + quad
    order = np.argsort(key, kind="stable")
    counts = np.bincount(key, minlength=NT * Q).reshape(NT, Q)
    starts = np.zeros((NT, Q), np.int64)
    starts.reshape(-1)[1:] = np.cumsum(counts.reshape(-1))[:-1]

    # rank-matched slot assignment: sort tiles by total count (desc), deal
    # rank r to core r%8, slot r//8 -> the 8 tiles at a slot have similar
    # counts, minimizing per-slot max padding.
    totals = counts.sum(1)
    rank = np.argsort(-totals, kind="stable")
    tiles_sc = rank.reshape(TPC, NCORES)           # [slot][core] -> tile id

    cnt_sc = counts[tiles_sc]                      # [slot][core][quad]
    K_sq = ((cnt_sc.max(axis=1) + P - 1) // P * P).astype(np.int64)  # [s][q]
    Csq = (K_sq // P).astype(np.int64)
    C_s = Csq.sum(1)
    S_s = K_sq.sum(1) // 16
    idx_cols = int(S_s.sum())
    chunk_cols = int(C_s.sum())

    if not EMULATE:
        key_prog = (tuple(map(tuple, K_sq)),)
        if key_prog not in _cache:
            _cache[key_prog] = _build_program(K_sq.tolist(), Csq.tolist())
        nc = _cache[key_prog]

    srcl_ord = srcl_all[order]
    dstl_ord = dstl_all[order]
    ew_ord = ew_all[order]

    in_maps = []
    for c in range(NCORES):
        idx16 = np.zeros((P, idx_cols), np.int16)
        dstl_a = np.zeros((P, chunk_cols), np.float32)
        ew_a = np.zeros((P, chunk_cols), np.float32)
        ioff = 0
        coff = 0
        for s in range(TPC):
            t = tiles_sc[s, c]
            for q in range(Q):
                K = int(K_sq[s, q])
                if K == 0:
                    continue
                cq = int(Csq[s, q])
                cnt = int(counts[t, q])
                st = int(starts[t, q])
                seg_src = np.zeros((K,), np.int16)
                seg_src[:cnt] = srcl_ord[st:st + cnt]
                seg_dstl = np.zeros((cq * P,), np.float32)
                seg_dstl[:cnt] = dstl_ord[st:st + cnt]
                seg_ew = np.zeros((cq * P,), np.float32)
                seg_ew[:cnt] = ew_ord[st:st + cnt]
                wrapped = seg_src.reshape(K // 16, 16).T      # [16, K/16]
                idx16[:, ioff:ioff + K // 16] = np.tile(wrapped, (8, 1))
                dstl_a[:, coff:coff + cq] = seg_dstl.reshape(cq, P).T
                ew_a[:, coff:coff + cq] = seg_ew.reshape(cq, P).T
                ioff += K // 16
                coff += cq
        node_ids = tiles_sc[:, c][:, None] * P + np.arange(P)[None, :]
        ncol = norm_pad[node_ids].T.astype(np.float32).copy()   # [128, TPC]
        in_maps.append({
            "h0": hq[0], "h1": hq[1], "h2": hq[2], "h3": hq[3],
            "idx16": idx16, "dstl": dstl_a, "ew": ew_a,
            "ncol": np.ascontiguousarray(ncol),
            "bb": np.tile(bias[None, :], (P, 1)).astype(np.float32),
            "wt": weight,
        })

    global LAST_NC, LAST_IN_MAPS
    LAST_NC, LAST_IN_MAPS = (nc if not EMULATE else None), in_maps
    if EMULATE:
        results = [_emulate_core(m, K_sq, Csq) for m in in_maps]
    else:
        res = run_bass_kernel_spmd(nc, in_maps, core_ids=list(range(NCORES)),
                                   **RUN_KWARGS)
        global LAST_RESULTS
        LAST_RESULTS = res
        results = [res.results[c]["out"] for c in range(NCORES)]

    out_tiles = np.zeros((NT, P, P), np.float32)
    for c in range(NCORES):
        out_tiles[tiles_sc[:, c]] = results[c].reshape(TPC, P, P)
    return out_tiles.reshape(NPAD, P)[:N].copy()


def _emulate_core(m, K_sq, Csq):
    """Numpy emulation of the device program (mirrors _build_program)."""
    hq = [m[f"h{q}"] for q in range(Q)]
    iota = np.arange(P, dtype=np.float32)[None, :]          # [1,128]
    out = np.zeros((TPC, P, P), np.float32)
    ioff = 0
    coff = 0
    for s in range(TPC):
        msgs = []
        for q in range(Q):
            K = int(K_sq[s, q])
            if K == 0:
                continue
            idx = m["idx16"][:16, ioff:ioff + K // 16].T.reshape(-1)  # unwrap
            g = hq[q][idx]                                  # [K,128]
            msgs.append(g.reshape(K // P, P, P))            # [cq, part, feat]
            ioff += K // 16
        msgs = np.concatenate(msgs, axis=0)                 # [C_s, 128, 128]
        aggT = np.zeros((P, P), np.float32)
        nchunks = int(Csq[s].sum())
        for c in range(nchunks):
            col = coff + c
            dstl = m["dstl"][:, col][:, None]               # [128,1]
            ew = m["ew"][:, col][:, None]
            pt = (iota == dstl).astype(np.float32) * ew     # [128 edge,128 node]
            aggT += msgs[c].T @ pt                          # [feat, node]
        coff += nchunks
        o = aggT.T @ m["wt"]                                # [node, feat]
        o = o * m["ncol"][:, s][:, None] + m["bb"]
        out[s] = o
    return out



# revision 6
# speedup vs baseline: 1.0219x; 1.0219x over previous
"""GCN layer on 8 trn2 NeuronCores.

out = segment_sum((h @ W * norm)[src], dst) * norm + bias
    = (segment_sum((h*norm)[src], dst) @ W) * norm + bias   (+ bias on host)

Per-core structure:
- h pre-scaled by norm on host, bf16, split into 4 quadrant tables so
  dma_gather's int16 indices can address them.
- 98 dst-tiles (slots) rank-dealt across the 8 cores (one shared SPMD
  program); slots grouped G=7 per gather group (NG=14), round-robin so
  groups are balanced.
- Per (slot, quad) edge segments pad to x16 only; per (group, quad) the
  concatenated segments pad to x128 and gather in <=1024-row pieces
  (SWDGE ring limit) -> ~224 gathers/core vs 392 in the per-slot scheme,
  with ~9% fewer gathered rows.
- A 128-row chunk may span multiple slots; each (chunk, slot) pair is a
  "task" with its own masked one-hot column (masked rows hold dstl=999,
  whose one-hot column is all-zero, so no per-edge weight is needed).
- All of a (g,q)'s one-hot columns build in ONE DVE tensor_tensor via a
  stride-0 broadcast AP against an iota row.
- Chunk matmuls (bf16, PSUM fp32) accumulate each slot's aggT in its own
  PSUM bank across the 4 quadrant sweeps; epilogue: Act-engine PSUM->SBUF
  copy (bf16), weight matmul into the recycled PSUM bank, Act-engine
  x norm scale, DMA out. bias is added on the host.
"""
import numpy as np

import concourse.mybir as mybir
import concourse.tile as tile
from concourse import bacc
from concourse.bass import broadcast_tensor_aps
from concourse.bass_utils import run_bass_kernel_spmd
from concourse.library_config import mlp

P = 128
N = 100000
E = 1600000
NCORES = 8
NT = 784
NPAD = NT * P
TPC = NT // NCORES      # 98 slots per core
Q = 4
R = NPAD // Q           # 25088 rows per quadrant table
G = 7                   # slots per gather group
SUBK = 1024             # max rows per dma_gather (SWDGE ring limit)
NG = TPC // G           # 14 groups

_cache = {}
RUN_KWARGS = {}
LAST_RESULTS = None
LAST_NC = None
LAST_IN_MAPS = None
EMULATE = False


def _group_slots():
    return [[g + j * NG for j in range(G)] for g in range(NG)]


def _layout(K16_sq):
    """Static per-core layout from the 16-padded per-(slot,quad) counts.

    Returns dict with:
      gq: list over (g, q) of dict(
          K: padded gather rows (x128), idx_off, nchunk,
          seg: list of (slot, row_start, nrows) within the gather,
          tasks: list of (pt_col_local, chunk, slot),
          pt_off: global dstl col offset, ncols)
      idx_cols, pt_cols, task index per slot etc.
    """
    groups = _group_slots()
    gq = []
    idx_off = 0
    pt_off = 0
    slot_ntasks = {s: 0 for s in range(TPC)}
    for g, slots in enumerate(groups):
        for q in range(Q):
            seg = []
            row = 0
            for s in slots:
                k = int(K16_sq[s][q])
                if k == 0:
                    continue
                seg.append((s, row, k))
                row += k
            K = (row + P - 1) // P * P
            nchunk = K // P
            tasks = []
            for s, r0, k in seg:
                c0 = r0 // P
                c1 = (r0 + k - 1) // P
                for c in range(c0, c1 + 1):
                    tasks.append((len(tasks), c, s))
                    slot_ntasks[s] += 1
            gq.append(dict(g=g, q=q, K=K, idx_off=idx_off, nchunk=nchunk,
                           seg=seg, tasks=tasks, pt_off=pt_off,
                           ncols=len(tasks)))
            idx_off += K // 16
            pt_off += len(tasks)
    return dict(gq=gq, idx_cols=idx_off, pt_cols=pt_off,
                slot_ntasks=slot_ntasks, groups=groups)


def _refine_slots(tiles_sc, counts):
    """Local search: swap tiles between nearby slots to reduce the summed
    per-(slot, quad) 16-padded max over the 8 cores (= gathered rows)."""
    slots = tiles_sc.copy()
    rng = np.random.default_rng(12345)

    def cost(tile_ids):
        c = counts[tile_ids]
        return int(((c.max(axis=0) + 15) // 16 * 16).sum())

    cur = np.array([cost(slots[s]) for s in range(TPC)], np.int64)
    for _ in range(60000):
        s1 = int(rng.integers(TPC))
        s2 = s1 + int(rng.integers(1, 5))
        if s2 >= TPC:
            continue
        i = int(rng.integers(NCORES))
        j = int(rng.integers(NCORES))
        a, b = slots[s1, i], slots[s2, j]
        slots[s1, i], slots[s2, j] = b, a
        c1, c2 = cost(slots[s1]), cost(slots[s2])
        if c1 + c2 < cur[s1] + cur[s2]:
            cur[s1], cur[s2] = c1, c2
        else:
            slots[s1, i], slots[s2, j] = a, b
    return slots


def _build_program(lay, repeat=1):
    f32 = mybir.dt.float32
    bf16 = mybir.dt.bfloat16
    nc = bacc.Bacc(None, target_bir_lowering=False)
    hq_d = [nc.dram_tensor(f"h{q}", [R, P], bf16, kind="ExternalInput")
            for q in range(Q)]
    idx_d = nc.dram_tensor("idx16", [P, lay["idx_cols"]], mybir.dt.int16,
                           kind="ExternalInput")
    dstl_d = nc.dram_tensor("dstl", [P, lay["pt_cols"]], bf16,
                            kind="ExternalInput")
    ncol_d = nc.dram_tensor("ncol", [P, TPC], f32, kind="ExternalInput")
    w_d = nc.dram_tensor("wt", [P, P], bf16, kind="ExternalInput")
    out_d = nc.dram_tensor("out", [TPC * P, P], f32, kind="ExternalOutput")
    out_v = out_d.rearrange("(t p) d -> t p d", p=P)

    groups = lay["groups"]
    gq_list = lay["gq"]
    slot_ntasks = lay["slot_ntasks"]

    with tile.TileContext(nc) as tc:
        with (
            tc.tile_pool(name="const", bufs=1) as cpool,
            tc.tile_pool(name="gather", bufs=3) as gpool,
            tc.tile_pool(name="pt", bufs=3) as ptpool,
            tc.tile_pool(name="ps", bufs=1, space="PSUM") as pspool,
            tc.tile_pool(name="oo", bufs=3) as opool,
            tc.tile_pool(name="agg", bufs=3) as aggpool,
        ):
            nc.gpsimd.load_library(mlp)
            idx_sb = cpool.tile([P, lay["idx_cols"]], mybir.dt.int16)
            nc.sync.dma_start(idx_sb[:], idx_d[:])
            dstl_sb = cpool.tile([P, lay["pt_cols"]], bf16)
            nc.sync.dma_start(dstl_sb[:], dstl_d[:])
            ncol_sb = cpool.tile([P, TPC], f32)
            nc.sync.dma_start(ncol_sb[:], ncol_d[:])
            w_sb = cpool.tile([P, P], bf16)
            nc.sync.dma_start(w_sb[:], w_d[:])
            iota_i = cpool.tile([P, P], mybir.dt.int32)
            nc.gpsimd.iota(iota_i[:], pattern=[[1, P]], base=0,
                           channel_multiplier=0)
            iota_f = cpool.tile([P, P], bf16)
            nc.vector.tensor_copy(iota_f[:], iota_i[:])

            for rep in range(repeat):
              for g, slots in enumerate(groups):
                agg_ps = {}
                for j, s in enumerate(slots):
                    if slot_ntasks[s] > 0:
                        agg_ps[s] = pspool.tile([P, P], f32, tag=f"agg{j}",
                                                name=f"agg_g{g}_{j}")
                done = {s: 0 for s in slots}
                for q in range(Q):
                    info = gq_list[g * Q + q]
                    if info["K"] == 0:
                        continue
                    nchunk = info["nchunk"]
                    msgs = gpool.tile([P, nchunk, P], bf16, tag="msgs")
                    io = info["idx_off"]
                    for k0 in range(0, info["K"], SUBK):
                        kl = min(SUBK, info["K"] - k0)
                        nc.gpsimd.dma_gather(
                            msgs[:, k0 // P:(k0 + kl) // P, :], hq_d[q][:],
                            idx_sb[:, io + k0 // 16:io + (k0 + kl) // 16],
                            kl, kl, P,
                        )
                    ncols = info["ncols"]
                    pt = ptpool.tile([P, ncols, P], bf16, tag="pt")
                    d_ap = dstl_sb[:, info["pt_off"]:info["pt_off"] + ncols]
                    b0, b1 = broadcast_tensor_aps(
                        iota_f[:].rearrange("p (c j) -> p c j", c=1),
                        d_ap.rearrange("p (c j) -> p c j", j=1),
                    )
                    nc.vector.tensor_tensor(pt[:], b0, b1,
                                            op=mybir.AluOpType.is_equal)
                    for col, c, s in info["tasks"]:
                        done[s] += 1
                        nc.tensor.matmul(
                            agg_ps[s][:], lhsT=msgs[:, c, :],
                            rhs=pt[:, col, :],
                            start=(done[s] == 1),
                            stop=(done[s] == slot_ntasks[s]),
                        )
                for j, s in enumerate(slots):
                    if slot_ntasks[s] == 0:
                        continue
                    aggT_sb = aggpool.tile([P, P], bf16, tag="aggT")
                    nc.scalar.copy(aggT_sb[:], agg_ps[s][:])
                    out_ps = pspool.tile([P, P], f32, tag=f"agg{j}",
                                         name=f"out_g{g}_{j}")
                    nc.tensor.matmul(out_ps[:], lhsT=aggT_sb[:], rhs=w_sb[:],
                                     start=True, stop=True)
                    o_sb = opool.tile([P, P], f32, tag="o")
                    nc.scalar.activation(
                        o_sb[:], out_ps[:], mybir.ActivationFunctionType.Copy,
                        scale=ncol_sb[:, s:s + 1],
                    )
                    nc.sync.dma_start(out_v[s], o_sb[:])
    nc.compile()
    return nc


def _to_bf16(a):
    import ml_dtypes
    return a.astype(ml_dtypes.bfloat16)


def _host_prep(h, norm, src, dst, weight, bias):
    h_scaled = (h * norm[:, None]).astype(np.float32)
    h_pad = np.zeros((NPAD, P), np.float32)
    h_pad[:N] = h_scaled
    hq = [_to_bf16(h_pad[q * R:(q + 1) * R]) for q in range(Q)]
    norm_pad = np.zeros((NPAD,), np.float32)
    norm_pad[:N] = norm

    tile_id = dst // P
    dstl_all = (dst % P).astype(np.float32)
    quad = src // R
    srcl_all = (src % R).astype(np.int16)

    key = tile_id * Q + quad
    order = np.argsort(key, kind="stable")
    counts = np.bincount(key, minlength=NT * Q).reshape(NT, Q)
    starts = np.zeros((NT, Q), np.int64)
    starts.reshape(-1)[1:] = np.cumsum(counts.reshape(-1))[:-1]

    totals = counts.sum(1)
    rank = np.argsort(-totals, kind="stable")
    tiles_sc = rank.reshape(TPC, NCORES)
    tiles_sc = _refine_slots(tiles_sc, counts)

    cnt_sc = counts[tiles_sc]                       # [slot][core][quad]
    K16_sq = ((cnt_sc.max(axis=1) + 15) // 16 * 16).astype(np.int64)

    lay = _layout(K16_sq.tolist())

    srcl_ord = srcl_all[order]
    dstl_ord = dstl_all[order]

    in_maps = []
    for c in range(NCORES):
        idx16 = np.zeros((P, lay["idx_cols"]), np.int16)
        dstl_a = np.full((P, lay["pt_cols"]), 999.0, np.float32)
        for info in lay["gq"]:
            q = info["q"]
            K = info["K"]
            if K == 0:
                continue
            rows_src = np.zeros((K,), np.int16)
            rows_dstl = np.full((K,), 999.0, np.float32)
            for s, r0, k in info["seg"]:
                t = tiles_sc[s, c]
                cnt = int(counts[t, q])
                st = int(starts[t, q])
                assert cnt <= k
                rows_src[r0:r0 + cnt] = srcl_ord[st:st + cnt]
                rows_dstl[r0:r0 + cnt] = dstl_ord[st:st + cnt]
            io = info["idx_off"]
            wrapped = rows_src.reshape(K // 16, 16).T
            idx16[:, io:io + K // 16] = np.tile(wrapped, (8, 1))
            # per-task masked dstl columns
            for col, ch, s in info["tasks"]:
                seg = next(x for x in info["seg"] if x[0] == s)
                _, r0, k = seg
                lo = max(r0, ch * P)
                hi = min(r0 + k, (ch + 1) * P)
                colv = np.full((P,), 999.0, np.float32)
                colv[lo - ch * P:hi - ch * P] = rows_dstl[lo:hi]
                dstl_a[:, info["pt_off"] + col] = colv
        node_ids = tiles_sc[:, c][:, None] * P + np.arange(P)[None, :]
        ncol = norm_pad[node_ids].T.astype(np.float32).copy()
        in_maps.append({
            "h0": hq[0], "h1": hq[1], "h2": hq[2], "h3": hq[3],
            "idx16": idx16, "dstl": _to_bf16(dstl_a),
            "ncol": np.ascontiguousarray(ncol),
            "wt": _to_bf16(weight),
        })
    return lay, in_maps, tiles_sc


def _emulate_core(m, lay):
    """Numpy mirror of the device program for one core."""
    hq = [m[f"h{q}"].astype(np.float32) for q in range(Q)]
    w = m["wt"].astype(np.float32)
    iota = np.arange(P, dtype=np.float32)
    dstl = m["dstl"].astype(np.float32)
    out = np.zeros((TPC, P, P), np.float32)
    groups = lay["groups"]
    slot_ntasks = lay["slot_ntasks"]
    for g, slots in enumerate(groups):
        aggT = {s: np.zeros((P, P), np.float32) for s in slots}
        for q in range(Q):
            info = lay["gq"][g * Q + q]
            if info["K"] == 0:
                continue
            K = info["K"]
            io = info["idx_off"]
            idx = m["idx16"][:16, io:io + K // 16].T.reshape(-1)
            msgs = hq[q][idx].reshape(K // P, P, P)      # [chunk, e, feat]
            for col, ch, s in info["tasks"]:
                dcol = dstl[:, info["pt_off"] + col]     # [128]
                pt = (iota[None, :] == dcol[:, None]).astype(np.float32)
                bfpt = _to_bf16(pt).astype(np.float32)
                aggT[s] += msgs[ch].T @ bfpt
        for s in slots:
            if slot_ntasks[s] == 0:
                continue
            aggT_bf = _to_bf16(aggT[s]).astype(np.float32)
            o = aggT_bf.T @ w
            o = o * m["ncol"][:, s][:, None]
            out[s] = o
    return out


def kernel(h, norm, src, dst, weight, bias):
    h = np.ascontiguousarray(h, dtype=np.float32)
    norm = np.ascontiguousarray(norm, dtype=np.float32).reshape(-1)
    src = np.ascontiguousarray(src, dtype=np.int64).reshape(-1)
    dst = np.ascontiguousarray(dst, dtype=np.int64).reshape(-1)
    weight = np.ascontiguousarray(weight, dtype=np.float32)
    bias = np.ascontiguousarray(bias, dtype=np.float32).reshape(-1)
    assert h.shape == (N, P) and src.shape[0] == E

    lay, in_maps, tiles_sc = _host_prep(h, norm, src, dst, weight, bias)

    if EMULATE:
        results = [_emulate_core(m, lay) for m in in_maps]
        out_tiles = np.zeros((NT, P, P), np.float32)
        for c in range(NCORES):
            out_tiles[tiles_sc[:, c]] = results[c]
        out = out_tiles.reshape(NPAD, P)[:N]
        return out + bias[None, :]

    key_prog = ("v3", tuple(i["K"] for i in lay["gq"]),
                tuple(i["ncols"] for i in lay["gq"]))
    if key_prog not in _cache:
        _cache[key_prog] = _build_program(lay)
    nc = _cache[key_prog]

    global LAST_NC, LAST_IN_MAPS, LAST_RESULTS
    LAST_NC, LAST_IN_MAPS = nc, in_maps
    res = run_bass_kernel_spmd(nc, in_maps, core_ids=list(range(NCORES)),
                               **RUN_KWARGS)
    LAST_RESULTS = res
    results = [res.results[c]["out"] for c in range(NCORES)]

    out_tiles = np.zeros((NT, P, P), np.float32)
    for c in range(NCORES):
        out_tiles[tiles_sc[:, c]] = results[c].reshape(TPC, P, P)
    out = out_tiles.reshape(NPAD, P)[:N]
    return out + bias[None, :]


# revision 7
# speedup vs baseline: 1.0233x; 1.0014x over previous
"""GCN layer on 8 trn2 NeuronCores.

out = segment_sum((h @ W * norm)[src], dst) * norm + bias
    = (segment_sum((h*norm)[src], dst) @ W) * norm + bias   (+ bias on host)

Per-core structure:
- h pre-scaled by norm on host, bf16, split into 4 quadrant tables so
  dma_gather's int16 indices can address them.
- 98 dst-tiles (slots) rank-dealt across the 8 cores (one shared SPMD
  program); slots grouped G=7 per gather group (NG=14), round-robin so
  groups are balanced.
- Per (slot, quad) edge segments pad to x16 only; per (group, quad) the
  concatenated segments pad to x128 and gather in <=1024-row pieces
  (SWDGE ring limit) -> ~224 gathers/core vs 392 in the per-slot scheme,
  with ~9% fewer gathered rows.
- A 128-row chunk may span multiple slots; each (chunk, slot) pair is a
  "task" with its own masked one-hot column (masked rows hold dstl=999,
  whose one-hot column is all-zero, so no per-edge weight is needed).
- All of a (g,q)'s one-hot columns build in ONE DVE tensor_tensor via a
  stride-0 broadcast AP against an iota row.
- Chunk matmuls (bf16, PSUM fp32) accumulate each slot's aggT in its own
  PSUM bank across the 4 quadrant sweeps; epilogue: Act-engine PSUM->SBUF
  copy (bf16), weight matmul into the recycled PSUM bank, Act-engine
  x norm scale, DMA out. bias is added on the host.
"""
import numpy as np

import concourse.mybir as mybir
import concourse.tile as tile
from concourse import bacc
from concourse.bass import broadcast_tensor_aps
from concourse.bass_utils import run_bass_kernel_spmd
from concourse.library_config import mlp

P = 128
N = 100000
E = 1600000
NCORES = 8
NT = 784
NPAD = NT * P
TPC = NT // NCORES      # 98 slots per core
Q = 4
R = NPAD // Q           # 25088 rows per quadrant table
G = 7                   # slots per gather group
SUBK = 1024             # max rows per dma_gather (SWDGE ring limit)
NG = TPC // G           # 14 groups

_cache = {}
RUN_KWARGS = {}
LAST_RESULTS = None
LAST_NC = None
LAST_IN_MAPS = None
EMULATE = False


def _group_slots():
    return [[g + j * NG for j in range(G)] for g in range(NG)]


def _layout(K16_sq):
    """Static per-core layout from the 16-padded per-(slot,quad) counts.

    Returns dict with:
      gq: list over (g, q) of dict(
          K: padded gather rows (x128), idx_off, nchunk,
          seg: list of (slot, row_start, nrows) within the gather,
          tasks: list of (pt_col_local, chunk, slot),
          pt_off: global dstl col offset, ncols)
      idx_cols, pt_cols, task index per slot etc.
    """
    groups = _group_slots()
    gq = []
    idx_off = 0
    pt_off = 0
    slot_ntasks = {s: 0 for s in range(TPC)}
    for g, slots in enumerate(groups):
        for q in range(Q):
            seg = []
            row = 0
            for s in slots:
                k = int(K16_sq[s][q])
                if k == 0:
                    continue
                seg.append((s, row, k))
                row += k
            K = (row + P - 1) // P * P
            nchunk = K // P
            tasks = []
            for s, r0, k in seg:
                c0 = r0 // P
                c1 = (r0 + k - 1) // P
                for c in range(c0, c1 + 1):
                    tasks.append((len(tasks), c, s))
                    slot_ntasks[s] += 1
            gq.append(dict(g=g, q=q, K=K, idx_off=idx_off, nchunk=nchunk,
                           seg=seg, tasks=tasks, pt_off=pt_off,
                           ncols=len(tasks)))
            idx_off += K // 16
            pt_off += len(tasks)
    return dict(gq=gq, idx_cols=idx_off, pt_cols=pt_off,
                slot_ntasks=slot_ntasks, groups=groups)


def _refine_slots(tiles_sc, counts):
    """Local search: swap tiles between nearby slots to reduce the summed
    per-(slot, quad) 16-padded max over the 8 cores (= gathered rows)."""
    slots = tiles_sc.copy()
    rng = np.random.default_rng(12345)

    def cost(tile_ids):
        c = counts[tile_ids]
        return int(((c.max(axis=0) + 15) // 16 * 16).sum())

    cur = np.array([cost(slots[s]) for s in range(TPC)], np.int64)
    for _ in range(60000):
        s1 = int(rng.integers(TPC))
        s2 = s1 + int(rng.integers(1, 5))
        if s2 >= TPC:
            continue
        i = int(rng.integers(NCORES))
        j = int(rng.integers(NCORES))
        a, b = slots[s1, i], slots[s2, j]
        slots[s1, i], slots[s2, j] = b, a
        c1, c2 = cost(slots[s1]), cost(slots[s2])
        if c1 + c2 < cur[s1] + cur[s2]:
            cur[s1], cur[s2] = c1, c2
        else:
            slots[s1, i], slots[s2, j] = a, b
    return slots


def _build_program(lay, repeat=1):
    f32 = mybir.dt.float32
    bf16 = mybir.dt.bfloat16
    nc = bacc.Bacc(None, target_bir_lowering=False)
    hq_d = [nc.dram_tensor(f"h{q}", [R, P], bf16, kind="ExternalInput")
            for q in range(Q)]
    idx_d = nc.dram_tensor("idx16", [P, lay["idx_cols"]], mybir.dt.int16,
                           kind="ExternalInput")
    dstl_d = nc.dram_tensor("dstl", [P, lay["pt_cols"]], bf16,
                            kind="ExternalInput")
    ncol_d = nc.dram_tensor("ncol", [P, TPC], f32, kind="ExternalInput")
    w_d = nc.dram_tensor("wt", [P, P], bf16, kind="ExternalInput")
    out_d = nc.dram_tensor("out", [TPC * P, P], bf16,
                       kind="ExternalOutput")
    out_v = out_d.rearrange("(t p) d -> t p d", p=P)

    groups = lay["groups"]
    gq_list = lay["gq"]
    slot_ntasks = lay["slot_ntasks"]

    with tile.TileContext(nc) as tc:
        with (
            tc.tile_pool(name="const", bufs=1) as cpool,
            tc.tile_pool(name="gather", bufs=3) as gpool,
            tc.tile_pool(name="pt", bufs=3) as ptpool,
            tc.tile_pool(name="ps", bufs=1, space="PSUM") as pspool,
            tc.tile_pool(name="oo", bufs=3) as opool,
            tc.tile_pool(name="agg", bufs=3) as aggpool,
        ):
            nc.gpsimd.load_library(mlp)
            idx_sb = cpool.tile([P, lay["idx_cols"]], mybir.dt.int16)
            dstl_sb = cpool.tile([P, lay["pt_cols"]], bf16)
            g0 = [i for i in gq_list if i["g"] == 0 and i["K"] > 0]
            i0 = max(i["idx_off"] + i["K"] // 16 for i in g0)
            p0 = max(i["pt_off"] + i["ncols"] for i in g0)
            nc.sync.dma_start(idx_sb[:, :i0], idx_d[:, :i0])
            nc.sync.dma_start(dstl_sb[:, :p0], dstl_d[:, :p0])
            nc.sync.dma_start(idx_sb[:, i0:], idx_d[:, i0:])
            nc.sync.dma_start(dstl_sb[:, p0:], dstl_d[:, p0:])
            ncol_sb = cpool.tile([P, TPC], f32)
            nc.sync.dma_start(ncol_sb[:], ncol_d[:])
            w_sb = cpool.tile([P, P], bf16)
            nc.sync.dma_start(w_sb[:], w_d[:])
            iota_i = cpool.tile([P, P], mybir.dt.int32)
            nc.gpsimd.iota(iota_i[:], pattern=[[1, P]], base=0,
                           channel_multiplier=0)
            iota_f = cpool.tile([P, P], bf16)
            nc.vector.tensor_copy(iota_f[:], iota_i[:])

            for rep in range(repeat):
              for g, slots in enumerate(groups):
                agg_ps = {}
                for j, s in enumerate(slots):
                    if slot_ntasks[s] > 0:
                        agg_ps[s] = pspool.tile([P, P], f32, tag=f"agg{j}",
                                                name=f"agg_g{g}_{j}")
                done = {s: 0 for s in slots}
                for q in range(Q):
                    info = gq_list[g * Q + q]
                    if info["K"] == 0:
                        continue
                    nchunk = info["nchunk"]
                    msgs = gpool.tile([P, nchunk, P], bf16, tag="msgs")
                    io = info["idx_off"]
                    for k0 in range(0, info["K"], SUBK):
                        kl = min(SUBK, info["K"] - k0)
                        nc.gpsimd.dma_gather(
                            msgs[:, k0 // P:(k0 + kl) // P, :], hq_d[q][:],
                            idx_sb[:, io + k0 // 16:io + (k0 + kl) // 16],
                            kl, kl, P,
                        )
                    ncols = info["ncols"]
                    pt = ptpool.tile([P, ncols, P], bf16, tag="pt")
                    d_ap = dstl_sb[:, info["pt_off"]:info["pt_off"] + ncols]
                    b0, b1 = broadcast_tensor_aps(
                        iota_f[:].rearrange("p (c j) -> p c j", c=1),
                        d_ap.rearrange("p (c j) -> p c j", j=1),
                    )
                    nc.vector.tensor_tensor(pt[:], b0, b1,
                                            op=mybir.AluOpType.is_equal)
                    for col, c, s in info["tasks"]:
                        done[s] += 1
                        nc.tensor.matmul(
                            agg_ps[s][:], lhsT=msgs[:, c, :],
                            rhs=pt[:, col, :],
                            start=(done[s] == 1),
                            stop=(done[s] == slot_ntasks[s]),
                        )
                for j, s in enumerate(slots):
                    if slot_ntasks[s] == 0:
                        continue
                    aggT_sb = aggpool.tile([P, P], bf16, tag="aggT")
                    nc.scalar.copy(aggT_sb[:], agg_ps[s][:])
                    out_ps = pspool.tile([P, P], f32, tag=f"agg{j}",
                                         name=f"out_g{g}_{j}")
                    nc.tensor.matmul(out_ps[:], lhsT=aggT_sb[:], rhs=w_sb[:],
                                     start=True, stop=True)
                    o_sb = opool.tile([P, P], bf16, tag="o")
                    nc.scalar.activation(
                        o_sb[:], out_ps[:], mybir.ActivationFunctionType.Copy,
                        scale=ncol_sb[:, s:s + 1],
                    )
                    nc.sync.dma_start(out_v[s], o_sb[:])
    nc.compile()
    return nc


def _to_bf16(a):
    import ml_dtypes
    return a.astype(ml_dtypes.bfloat16)


def _host_prep(h, norm, src, dst, weight, bias):
    h_scaled = (h * norm[:, None]).astype(np.float32)
    h_pad = np.zeros((NPAD, P), np.float32)
    h_pad[:N] = h_scaled
    hq = [_to_bf16(h_pad[q * R:(q + 1) * R]) for q in range(Q)]
    norm_pad = np.zeros((NPAD,), np.float32)
    norm_pad[:N] = norm

    tile_id = dst // P
    dstl_all = (dst % P).astype(np.float32)
    quad = src // R
    srcl_all = (src % R).astype(np.int16)

    key = tile_id * Q + quad
    order = np.argsort(key, kind="stable")
    counts = np.bincount(key, minlength=NT * Q).reshape(NT, Q)
    starts = np.zeros((NT, Q), np.int64)
    starts.reshape(-1)[1:] = np.cumsum(counts.reshape(-1))[:-1]

    totals = counts.sum(1)
    rank = np.argsort(-totals, kind="stable")
    tiles_sc = rank.reshape(TPC, NCORES)
    tiles_sc = _refine_slots(tiles_sc, counts)

    cnt_sc = counts[tiles_sc]                       # [slot][core][quad]
    K16_sq = ((cnt_sc.max(axis=1) + 15) // 16 * 16).astype(np.int64)

    lay = _layout(K16_sq.tolist())

    srcl_ord = srcl_all[order]
    dstl_ord = dstl_all[order]

    in_maps = []
    for c in range(NCORES):
        idx16 = np.zeros((P, lay["idx_cols"]), np.int16)
        dstl_a = np.full((P, lay["pt_cols"]), 999.0, np.float32)
        for info in lay["gq"]:
            q = info["q"]
            K = info["K"]
            if K == 0:
                continue
            rows_src = np.zeros((K,), np.int16)
            rows_dstl = np.full((K,), 999.0, np.float32)
            for s, r0, k in info["seg"]:
                t = tiles_sc[s, c]
                cnt = int(counts[t, q])
                st = int(starts[t, q])
                assert cnt <= k
                rows_src[r0:r0 + cnt] = srcl_ord[st:st + cnt]
                rows_dstl[r0:r0 + cnt] = dstl_ord[st:st + cnt]
            io = info["idx_off"]
            wrapped = rows_src.reshape(K // 16, 16).T
            idx16[:, io:io + K // 16] = np.tile(wrapped, (8, 1))
            # per-task masked dstl columns
            for col, ch, s in info["tasks"]:
                seg = next(x for x in info["seg"] if x[0] == s)
                _, r0, k = seg
                lo = max(r0, ch * P)
                hi = min(r0 + k, (ch + 1) * P)
                colv = np.full((P,), 999.0, np.float32)
                colv[lo - ch * P:hi - ch * P] = rows_dstl[lo:hi]
                dstl_a[:, info["pt_off"] + col] = colv
        node_ids = tiles_sc[:, c][:, None] * P + np.arange(P)[None, :]
        ncol = norm_pad[node_ids].T.astype(np.float32).copy()
        in_maps.append({
            "h0": hq[0], "h1": hq[1], "h2": hq[2], "h3": hq[3],
            "idx16": idx16, "dstl": _to_bf16(dstl_a),
            "ncol": np.ascontiguousarray(ncol),
            "wt": _to_bf16(weight),
        })
    return lay, in_maps, tiles_sc


def _emulate_core(m, lay):
    """Numpy mirror of the device program for one core."""
    hq = [m[f"h{q}"].astype(np.float32) for q in range(Q)]
    w = m["wt"].astype(np.float32)
    iota = np.arange(P, dtype=np.float32)
    dstl = m["dstl"].astype(np.float32)
    out = np.zeros((TPC, P, P), np.float32)
    groups = lay["groups"]
    slot_ntasks = lay["slot_ntasks"]
    for g, slots in enumerate(groups):
        aggT = {s: np.zeros((P, P), np.float32) for s in slots}
        for q in range(Q):
            info = lay["gq"][g * Q + q]
            if info["K"] == 0:
                continue
            K = info["K"]
            io = info["idx_off"]
            idx = m["idx16"][:16, io:io + K // 16].T.reshape(-1)
            msgs = hq[q][idx].reshape(K // P, P, P)      # [chunk, e, feat]
            for col, ch, s in info["tasks"]:
                dcol = dstl[:, info["pt_off"] + col]     # [128]
                pt = (iota[None, :] == dcol[:, None]).astype(np.float32)
                bfpt = _to_bf16(pt).astype(np.float32)
                aggT[s] += msgs[ch].T @ bfpt
        for s in slots:
            if slot_ntasks[s] == 0:
                continue
            aggT_bf = _to_bf16(aggT[s]).astype(np.float32)
            o = aggT_bf.T @ w
            o = o * m["ncol"][:, s][:, None]
            out[s] = _to_bf16(o).astype(np.float32)
    return out


def kernel(h, norm, src, dst, weight, bias):
    h = np.ascontiguousarray(h, dtype=np.float32)
    norm = np.ascontiguousarray(norm, dtype=np.float32).reshape(-1)
    src = np.ascontiguousarray(src, dtype=np.int64).reshape(-1)
    dst = np.ascontiguousarray(dst, dtype=np.int64).reshape(-1)
    weight = np.ascontiguousarray(weight, dtype=np.float32)
    bias = np.ascontiguousarray(bias, dtype=np.float32).reshape(-1)
    assert h.shape == (N, P) and src.shape[0] == E

    lay, in_maps, tiles_sc = _host_prep(h, norm, src, dst, weight, bias)

    if EMULATE:
        results = [_emulate_core(m, lay) for m in in_maps]
        out_tiles = np.zeros((NT, P, P), np.float32)
        for c in range(NCORES):
            out_tiles[tiles_sc[:, c]] = results[c]
        out = out_tiles.reshape(NPAD, P)[:N]
        return out + bias[None, :]

    key_prog = ("v3", tuple(i["K"] for i in lay["gq"]),
                tuple(i["ncols"] for i in lay["gq"]))
    if key_prog not in _cache:
        _cache[key_prog] = _build_program(lay)
    nc = _cache[key_prog]

    global LAST_NC, LAST_IN_MAPS, LAST_RESULTS
    LAST_NC, LAST_IN_MAPS = nc, in_maps
    res = run_bass_kernel_spmd(nc, in_maps, core_ids=list(range(NCORES)),
                               **RUN_KWARGS)
    LAST_RESULTS = res
    results = [res.results[c]["out"] for c in range(NCORES)]

    out_tiles = np.zeros((NT, P, P), np.float32)
    for c in range(NCORES):
        out_tiles[tiles_sc[:, c]] = results[c].astype(np.float32).reshape(
            TPC, P, P)
    out = out_tiles.reshape(NPAD, P)[:N]
    return out + bias[None, :]
